# revision 6
# baseline (speedup 1.0000x reference)
"""Trainium2 Bass kernel for nn_Attention_68676527063657 (v2).

Full multi-head attention with anti-causal mask (visible iff k > q):
  qw = q @ Wq.T; kw = k @ Wk.T; vw = v @ Wv.T   (per-head, dk=dv=64)
  a  = (qw . kw)/8 - (1-v_mask)*1e10 - tril(ones)*1e10
  o  = softmax(a) @ vw, then o *= q_mask

Sharding: core c in [0,8): batch b = c//4, head-group g = c%4 (4 heads each).
Each core computes o[b, :, 256g:256g+256]; host gathers.

v2 changes vs baseline:
  - fine-grained causal skip: for q-block j and k-chunk c = 4j+dd (dd<4)
    only the first W = 128*(dd+1) q-columns are visible; scores/exp/PV
    all run at width W. PV accumulators pre-zeroed (gpsimd memset) so
    ascending chunks accumulate with start=False.
  - local [128,2,128] strict-lower-triangle mask (one DVE mul per diag
    chunk, both heads at once) instead of 4 wide 512-col masks.
  - prologue: k-proj then q-proj(es0,j0) then attention starts; the
    remaining projections are emitted as fillers interleaved into the
    attention stream (deadline-scheduled).
  - inputs DMA'd in column pieces ordered so compute starts ~4us in.
  - one PSUM ring: tag "st" [128,2,512]f32 x3 (scores, proj psums,
    transpose batches) + tag "ot" [66,512]f32 x2 (PV accumulators).
  - output drained per (es, qb) column-half as soon as each half is done.
"""

import numpy as np

B, L, D = 2, 2048, 1024
H, DK = 16, 64
HG = 4            # heads per core
E = HG * DK       # 256 per-core output features
NCORES = 8
J, QB = 4, 512    # q blocks
C, KB = 16, 128   # k chunks
BIG = 1e10

_CACHE = {}
PROFILE = False
LAST_EXEC_NS = None
LAST_TRACE = None


def _build_program(nd, degen_qbs):
    import concourse.bass as bass
    import concourse.mybir as mybir
    from concourse import bacc
    from concourse.tile import TileContext

    F32 = mybir.dt.float32
    BF16 = mybir.dt.bfloat16
    AF = mybir.ActivationFunctionType
    ALU = mybir.AluOpType
    ts = bass.ts

    nc = bacc.Bacc(None)
    xq = nc.dram_tensor("xq", [D, L], BF16, kind="ExternalInput")
    xk = nc.dram_tensor("xk", [D, L], BF16, kind="ExternalInput")
    xv = nc.dram_tensor("xv", [D, L], BF16, kind="ExternalInput")
    wq = nc.dram_tensor("wq", [128, 8 * E], BF16, kind="ExternalInput")
    wk = nc.dram_tensor("wk", [128, 8 * E], BF16, kind="ExternalInput")
    wv = nc.dram_tensor("wv", [128, 8 * E], BF16, kind="ExternalInput")
    idn = nc.dram_tensor("idn", [128, 128], BF16, kind="ExternalInput")
    oz4 = nc.dram_tensor("oz4", [128, HG, 2], BF16, kind="ExternalInput")
    t01 = nc.dram_tensor("t01", [128, 2, 128], BF16, kind="ExternalInput")
    vmb = nc.dram_tensor("vmb", [128, C], F32, kind="ExternalInput")
    qmt = nc.dram_tensor("qmt", [128, C], F32, kind="ExternalInput")
    isdt = nc.dram_tensor("isdt", [128, C], F32, kind="ExternalInput")
    isdq = nc.dram_tensor("isdq", [128, C, nd], F32, kind="ExternalInput")
    wfx = nc.dram_tensor("wfx", [128, C, 2 * nd], BF16, kind="ExternalInput")
    o_d = nc.dram_tensor("o", [L, E], F32, kind="ExternalOutput")

    with TileContext(nc) as tc:
        with tc.tile_pool(name="consts", bufs=1) as consts, \
             tc.tile_pool(name="xk_p", bufs=8) as xkp, \
             tc.tile_pool(name="xq_p", bufs=8) as xqp, \
             tc.tile_pool(name="xv_p", bufs=8) as xvp, \
             tc.tile_pool(name="qk2", bufs=1) as qk2p, \
             tc.tile_pool(name="pp", bufs=8) as ppool, \
             tc.tile_pool(name="osb", bufs=2) as osbp, \
             tc.tile_pool(name="oall", bufs=1) as oallp, \
             tc.tile_pool(name="small", bufs=4) as small, \
             tc.tile_pool(name="fbp", bufs=1) as fbp, \
             tc.tile_pool(name="ps", bufs=1, space="PSUM") as psp:

            # x tiles + weights, DMA'd in compute-feed order
            xt_k = [xkp.tile([128, L], BF16, tag="xk", name=f"xtk_{d}")
                    for d in range(8)]
            xt_q = [xqp.tile([128, L], BF16, tag="xq", name=f"xtq_{d}")
                    for d in range(8)]
            xt_v = [xvp.tile([128, L], BF16, tag="xv", name=f"xtv_{d}")
                    for d in range(8)]
            wsb = {}

            def dma_w(nm):
                wdram = {"k": wk, "q": wq, "v": wv}[nm]
                wsb[nm] = consts.tile([128, 8, E], BF16, tag=f"w_{nm}",
                                      name=f"wsb_{nm}")
                nc.sync.dma_start(
                    out=wsb[nm][:, :, :],
                    in_=wdram.rearrange("p (t e) -> p t e", t=8))

            def dma_x(xin, tiles, half):
                for d in range(8):
                    nc.sync.dma_start(out=tiles[d][:, ts(half, 1024)],
                                      in_=xin[ts(d, 128), ts(half, 1024)])

            dma_w("k")
            dma_x(xk, xt_k, 0)                # xk cols 0:1024 (k-proj lc 0,1)
            dma_w("q")
            dma_x(xq, xt_q, 0)                # xq first half (q-proj j0, j1)

            idt = consts.tile([128, 128], BF16, tag="idt")
            nc.sync.dma_start(out=idt[:, :], in_=idn[:, :])
            t01t = consts.tile([128, 2, 128], BF16, tag="t01t")
            nc.sync.dma_start(out=t01t[:, :, :], in_=t01[:, :, :])
            vmbt = consts.tile([128, C], F32, tag="vmbt")
            nc.sync.dma_start(out=vmbt[:, :], in_=vmb[:, :])
            qmtt = consts.tile([128, C], F32, tag="qmtt")
            nc.sync.dma_start(out=qmtt[:, :], in_=qmt[:, :])
            oz4t = consts.tile([128, HG, 2], BF16, tag="oz4t")
            nc.sync.dma_start(out=oz4t[:, :, :], in_=oz4[:, :, :])

            dma_x(xk, xt_k, 1)                # xk cols 1024: (k02/k03 filler)
            dma_w("v")
            dma_x(xv, xt_v, 0)                # xv first half (vproj lt 0-7)
            dma_x(xq, xt_q, 1)
            dma_x(xv, xt_v, 1)

            # late consts (first needed at yield >= 15)
            isdtt = consts.tile([128, C], F32, tag="isdtt")
            nc.sync.dma_start(out=isdtt[:, :], in_=isdt[:, :])
            isdqt = consts.tile([128, C, nd], F32, tag="isdqt")
            nc.sync.dma_start(out=isdqt[:, :, :], in_=isdq[:, :, :])
            wfxt = consts.tile([128, C, 2 * nd], BF16, tag="wfxt")
            nc.sync.dma_start(out=wfxt[:, :, :], in_=wfx[:, :, :])

            # ---------------- persistent activation tiles ---------------------
            qw2 = [[qk2p.tile([128, QB], BF16, tag=f"qw2_{es}_{lc}",
                              name=f"qw2_{es}_{lc}") for lc in range(4)]
                   for es in range(2)]
            kw2 = [[qk2p.tile([128, QB], BF16, tag=f"kw2_{es}_{lc}",
                              name=f"kw2_{es}_{lc}") for lc in range(4)]
                   for es in range(2)]
            vw_c = [qk2p.tile([128, HG, 66], BF16, tag=f"vw_{c}",
                              name=f"vw_{c}") for c in range(C)]
            oallB = oallp.tile([128, C, E], F32, tag="oall", name="oallB")
            oall = [oallB[:, qb, :] for qb in range(C)]

            # ---------------- proj emitters (psum from shared "st" ring) ------
            def emit_qkproj(nm, es, lc, on_scalar):
                """one (es, lc) projection group: 8 matmuls + copy out."""
                ps = psp.tile([128, QB], F32, tag="pr", bufs=2,
                              name=f"pr_{nm}_{es}_{lc}")
                xt = xt_k if nm == "k" else xt_q
                for d in range(8):
                    nc.tensor.matmul(
                        ps, wsb[nm][:, d, ts(es, 128)],
                        xt[d][:, ts(lc, QB)],
                        start=(d == 0), stop=(d == 7))
                dst = (kw2 if nm == "k" else qw2)[es][lc]
                eng = nc.scalar if on_scalar else nc.vector
                if on_scalar:
                    eng.copy(out=dst[:, :], in_=ps)
                else:
                    eng.tensor_copy(out=dst[:, :], in_=ps)

            def emit_vproj(lt):
                slot = psp.tile([128, QB], F32, tag="pr", bufs=2,
                                name=f"vp_{lt}")
                ps = slot[:, 0:E]
                for d in range(8):
                    nc.tensor.matmul(
                        ps, xt_v[d][:, ts(lt, 128)], wsb["v"][:, d, :],
                        start=(d == 0), stop=(d == 7))
                nc.vector.tensor_copy(
                    out=vw_c[lt][:, :, 0:64],
                    in_=ps.rearrange("p (h e) -> p h e", h=HG))
                nc.sync.dma_start(out=vw_c[lt][:, :, 64:66], in_=oz4t[:, :, :])

            def emit_fix():
                fb = [[None] * nd for _ in range(HG)]
                for i in range(nd):
                    for h in range(HG):
                        pf = psp.tile([2, 64], F32, tag="pr", bufs=2,
                                      name=f"pf_{i}_{h}")
                        for c in range(C):
                            nc.tensor.matmul(
                                pf[:, :],
                                wfxt[:, c, 2 * i:2 * i + 2],
                                vw_c[c][:, h, 0:64],
                                start=(c == 0), stop=(c == C - 1))
                        fr = small.tile([1, 64], F32, tag="fixrow")
                        nc.vector.tensor_copy(out=fr[:, :], in_=pf[0:1, :])
                        t = fbp.tile([128, 64], F32, tag=f"fb_{h}_{i}")
                        nc.gpsimd.partition_broadcast(t[:, :], fr[0:1, :])
                        fb[h][i] = t
                return fb

            # two persistent p0 tiles for each block's first chunk: tails
            # [128:QB) are zeroed once and never rewritten (per-block exp
            # only writes cols [0:128)), so the first PV matmul can run
            # full-width with start=True and zero the whole accumulator
            # bank. Persistent tiles keep all ordering on one logical tile.
            p0_tiles = [qk2p.tile([128, 2, QB], BF16, tag=f"p0_{i}",
                                  name=f"p0_{i}") for i in range(2)]
            for i in range(2):
                nc.vector.memset(p0_tiles[i][:, :, :], 0.0)
            blk_counter = [0]

            # ---------------- attention stream -------------------------------
            LAG = 5
            fb_holder = {}

            def attention_stream():
                for es in range(2):
                    for j in range(J):
                        chunks = list(range(4 * j, C))
                        m = len(chunks)
                        wid = [min(128 * (c - 4 * j + 1), QB) for c in chunks]
                        ot2 = [psp.tile([66, QB], F32, tag="ot", bufs=2,
                                        name=f"ot_{es}_{j}_{s2}")
                               for s2 in range(2)]
                        pbuf = [None] * m

                        def emit_ot(idx, ot2=ot2, pbuf=pbuf, m=m, j=j, es=es,
                                    chunks=chunks, wid=wid):
                            c, w = chunks[idx], wid[idx]
                            last = idx == m - 1
                            for sub in range(2):
                                vws = vw_c[c][:, 2 * es + sub, :]
                                if idx == 0:
                                    # first chunk: full-width start=True;
                                    # pbuf is the p0 tile whose tail
                                    # [128:QB] is permanently zero, so
                                    # cols >= 128 get zeroed for the
                                    # later accumulating chunks.
                                    nc.tensor.matmul(
                                        ot2[sub][:, :], vws,
                                        pbuf[0][:, sub, :],
                                        start=True, stop=False,
                                        skip_group_check=True)
                                else:
                                    nc.tensor.matmul(
                                        ot2[sub][:, 0:w], vws,
                                        pbuf[idx][:, sub, 0:w],
                                        start=False, stop=last,
                                        skip_group_check=True)

                        for idx, c in enumerate(chunks):
                            w = wid[idx]
                            st2 = psp.tile([128, 2, QB], F32, tag="st", bufs=2,
                                           name=f"st_{es}_{j}_{c}")
                            for sub in range(2):
                                r0 = 64 * sub
                                nc.tensor.matmul(
                                    st2[:, sub, 0:w],
                                    kw2[es][c // 4][r0:r0 + 64, ts(c % 4, 128)],
                                    qw2[es][j][r0:r0 + 64, 0:w],
                                    start=True, stop=True)
                            if idx == 0:
                                p = p0_tiles[blk_counter[0] % 2]
                                blk_counter[0] += 1
                            else:
                                p = ppool.tile([128, 2, QB], BF16, tag="p")
                            nc.scalar.activation(
                                out=p[:, :, 0:w], in_=st2[:, :, 0:w],
                                func=AF.Exp,
                                bias=vmbt[:, c:c + 1], scale=0.125)
                            dd = c - 4 * j
                            if dd < 4:
                                off = w - 128
                                nc.vector.tensor_mul(
                                    p[:, :, off:off + 128],
                                    p[:, :, off:off + 128],
                                    t01t[:, :, :])
                            pbuf[idx] = p
                            if idx >= LAG:
                                emit_ot(idx - LAG)
                            yield
                        for idx in range(max(0, m - LAG), m):
                            emit_ot(idx)

                        for sub in range(2):
                            h = 2 * es + sub
                            osb = osbp.tile([66, QB], BF16, tag="osb")
                            nc.vector.tensor_copy(out=osb[:, :],
                                                  in_=ot2[sub][:, :])
                            for t in range(4):
                                qb = 4 * j + t
                                tr = psp.tile([128, 66], BF16, tag="pr",
                                              bufs=2, name=f"tr_{es}_{j}_{sub}_{t}")
                                nc.tensor.transpose(
                                    tr, osb[:, ts(t, 128)], idt[0:66, 0:66])
                                rc = small.tile([128, 1], F32, tag="rc")
                                if qb in degen_qbs:
                                    dn = small.tile([128, 1], F32, tag="dn")
                                    nc.vector.tensor_add(
                                        dn[:, :], tr[:, 64:65],
                                        isdtt[:, qb:qb + 1])
                                    nc.vector.reciprocal(rc[:, :], dn[:, :])
                                else:
                                    nc.vector.reciprocal(rc[:, :], tr[:, 64:65])
                                if es == 1 and j >= 2:
                                    # tail blocks: scale on the (now idle)
                                    # scalar engine to unclog the DVE chain
                                    rc2 = small.tile([128, 1], F32, tag="rc2")
                                    nc.vector.tensor_mul(
                                        rc2[:, :], rc[:, :],
                                        qmtt[:, qb:qb + 1])
                                    nc.scalar.activation(
                                        out=oall[qb][:, ts(h, 64)],
                                        in_=tr[:, 0:64], func=AF.Copy,
                                        scale=rc2[:, 0:1])
                                else:
                                    nc.vector.tensor_scalar(
                                        out=oall[qb][:, ts(h, 64)],
                                        in0=tr[:, 0:64], scalar1=rc[:, 0:1],
                                        scalar2=qmtt[:, qb:qb + 1],
                                        op0=ALU.mult, op1=ALU.mult)
                                for i in degen_qbs.get(qb, ()):
                                    fb = fb_holder["fb"]
                                    nc.vector.scalar_tensor_tensor(
                                        out=oall[qb][:, ts(h, 64)],
                                        in0=fb[h][i][:, :],
                                        scalar=isdqt[:, qb, i:i + 1],
                                        in1=oall[qb][:, ts(h, 64)],
                                        op0=ALU.mult, op1=ALU.add)
                            if sub == 1:
                                nc.sync.dma_start(
                                    out=o_d[ts(j, QB), ts(es, 128)].rearrange(
                                        "(t p) e -> p t e", p=128),
                                    in_=oallB[:, 4 * j:4 * j + 4, ts(es, 128)])
                        yield

            # ---------------- prologue + drive -------------------------------
            # prologue: k-proj lc0/lc1 (xk first half) + q-proj (0,0); the
            # rest of the projections are fillers inside the attention stream.
            emit_qkproj("k", 0, 0, on_scalar=True)
            emit_qkproj("k", 0, 1, on_scalar=True)
            emit_qkproj("q", 0, 0, on_scalar=True)

            F = lambda nm, es, lc, sc=False: (
                lambda: emit_qkproj(nm, es, lc, on_scalar=sc))
            # yield-indexed schedule. yields: chunks + 1 epilogue per block;
            # block starts: es0 j0@0 j1@17 j2@30 j3@39; es1 j0@44 j1@61
            # j2@74 j3@83. q-es1 fillers deferred into exp-bound es1 blocks.
            sched = {
                0: [F("q", 0, 1, True)],
                2: [F("k", 0, 2, True)], 4: [F("k", 0, 3, True)],
                6: [F("k", 1, 0, True)], 8: [F("k", 1, 1, True)],
                10: [F("k", 1, 2)], 12: [F("k", 1, 3)],
                14: [F("q", 0, 2)], 17: [F("q", 0, 3)],
                20: [F("q", 1, 0)],
                46: [F("q", 1, 1)], 63: [F("q", 1, 2)], 76: [F("q", 1, 3)],
            }
            # the fix tables (fb) are read at the es0 epilogue of any block
            # holding a degenerate row; emit_fix needs all 16 vw_c tiles.
            vdelay = 0
            # vproj(lt) at yield lt+vdelay (matches xv DMA arrival); PV(c)
            # needs vw_c at yield c+LAG
            for lt in range(C):
                sched.setdefault(lt + vdelay, []).append(
                    (lambda l: lambda: emit_vproj(l))(lt))
            sched.setdefault(C - 1 + vdelay + 1, []).append(
                lambda: fb_holder.update(fb=emit_fix()))

            stream = attention_stream()
            y = 0
            while True:
                for fn in sched.pop(y, ()):
                    fn()
                if next(stream, StopIteration) is StopIteration:
                    break
                y += 1
            for yy in sorted(sched):
                for fn in sched[yy]:
                    fn()
    nc.finalize()
    return nc


def _host_prep(q, k, v, v_mask, q_mask, Wq, Wk, Wv):
    """Per-core input maps + degenerate-row bookkeeping."""
    import ml_dtypes
    bf16 = ml_dtypes.bfloat16
    f32 = np.float32
    q, k, v = (np.asarray(x, f32) for x in (q, k, v))
    v_mask, q_mask = np.asarray(v_mask, f32), np.asarray(q_mask, f32)
    Wq, Wk, Wv = (np.asarray(x, f32) for x in (Wq, Wk, Wv))

    idn = np.eye(128, dtype=f32)
    oz4 = np.zeros((128, HG, 2), f32)
    oz4[:, :, 0] = 1.0
    # local strict-lower-triangle keep mask: keep x_local < p, dup'd on dim1
    p_i = np.arange(128)[:, None]
    x_i = np.arange(128)[None, :]
    t01 = np.where(x_i < p_i, f32(1.0), f32(0.0))
    t01 = np.broadcast_to(t01[:, None, :], (128, 2, 128)).copy()

    # degenerate rows per batch + fix weight vectors
    deg, wfix_cols = [], []
    for b in range(B):
        vm = v_mask[b]
        rows = [qq for qq in range(L)
                if qq == L - 1 or not vm[qq + 1:].any()]
        deg.append(rows)
        cols = []
        for qq in rows:
            single = np.zeros(L, f32)
            kk = np.arange(L)
            causal = kk <= qq
            pen = causal.astype(np.int64) + (vm == 0).astype(np.int64)
            m = pen == pen.min()   # max-attaining set under -BIG penalties
            single[m] = 1.0 / m.sum()
            cols.append(single)
        wfix_cols.append(cols)

    nd = max(len(r) for r in deg)
    degen_qbs = {}
    for b in range(B):
        for i, qq in enumerate(deg[b]):
            degen_qbs.setdefault(qq // 128, set()).add(i)
    degen_qbs = {qb: sorted(s) for qb, s in degen_qbs.items()}

    WqT, WkT, WvT = Wq.T.copy(), Wk.T.copy(), Wv.T.copy()
    in_maps = []
    for core in range(NCORES):
        b, g = divmod(core, HG)
        sl = slice(E * g, E * g + E)
        vm, qm = v_mask[b], q_mask[b]
        vmb = (-BIG * (1.0 - vm)).reshape(C, 128).T.astype(f32)
        qmt = qm.reshape(C, 128).T.astype(f32)
        isd_v = np.zeros(L, f32)
        isdq_v = np.zeros((L, nd), f32)
        wfx_v = np.zeros((L, 2 * nd), f32)
        for i, qq in enumerate(deg[b]):
            isd_v[qq] = 1.0
            isdq_v[qq, i] = qm[qq]
            wfx_v[:, 2 * i] = wfix_cols[b][i]
        in_maps.append({
            "xq": np.ascontiguousarray(q[b].T.astype(bf16)),
            "xk": np.ascontiguousarray(k[b].T.astype(bf16)),
            "xv": np.ascontiguousarray(v[b].T.astype(bf16)),
            # dense pack [128, 8*E]: row p holds [d-chunk, out-feature]
            "wq": np.ascontiguousarray(
                WqT[:, sl].reshape(8, 128, E).transpose(1, 0, 2)
                .reshape(128, 8 * E).astype(bf16)),
            "wk": np.ascontiguousarray(
                WkT[:, sl].reshape(8, 128, E).transpose(1, 0, 2)
                .reshape(128, 8 * E).astype(bf16)),
            "wv": np.ascontiguousarray(
                WvT[:, sl].reshape(8, 128, E).transpose(1, 0, 2)
                .reshape(128, 8 * E).astype(bf16)),
            "idn": idn.astype(bf16),
            "oz4": oz4.astype(bf16),
            "t01": t01.astype(bf16),
            "vmb": vmb, "qmt": qmt,
            "isdt": isd_v.reshape(C, 128).T.copy(),
            "isdq": np.ascontiguousarray(
                isdq_v.reshape(C, 128, nd).transpose(1, 0, 2)),
            "wfx": np.ascontiguousarray(
                wfx_v.reshape(C, 128, 2 * nd).transpose(1, 0, 2).astype(bf16)),
        })
    return in_maps, nd, degen_qbs


def kernel(q, k, v, v_mask, q_mask, Wq, Wk, Wv):
    global LAST_EXEC_NS, LAST_TRACE
    from concourse.bass_utils import run_bass_kernel_spmd

    in_maps, nd, degen_qbs = _host_prep(q, k, v, v_mask, q_mask, Wq, Wk, Wv)
    key = (nd,
           tuple(sorted((qb, tuple(i)) for qb, i in degen_qbs.items())))
    if key not in _CACHE:
        _CACHE[key] = _build_program(nd, degen_qbs)
    nc = _CACHE[key]

    kwargs = {}
    if PROFILE:
        import sys, types
        sys.path.insert(0, "/root/.axon_site/trn_agent_boot")
        import trn_boot
        raw = trn_boot._ntff_profile_via_ctypes("/opt/axon/libaxon_pjrt.so")
        mod = types.ModuleType("antenv.axon_hooks")
        mod.get_axon_ntff_profile_hook = (
            lambda: (lambda out_dir, ids: raw(out_dir, None)))
        sys.modules["antenv.axon_hooks"] = mod
        kwargs = dict(trace=True)

    res = run_bass_kernel_spmd(nc, in_maps, core_ids=list(range(NCORES)), **kwargs)
    if PROFILE:
        LAST_EXEC_NS = res.exec_time_ns
        LAST_TRACE = (res.instructions_and_trace[1]
                      if res.instructions_and_trace else None)

    out = np.empty((B, L, H * DK), np.float32)
    for core in range(NCORES):
        b, g = divmod(core, HG)
        out[b, :, E * g:E * g + E] = res.results[core]["o"]
    return out


# revision 8
# speedup vs baseline: 1.1478x; 1.1478x over previous
"""Trainium2 Bass kernel for nn_Attention_68676527063657 (v2).

Full multi-head attention with anti-causal mask (visible iff k > q):
  qw = q @ Wq.T; kw = k @ Wk.T; vw = v @ Wv.T   (per-head, dk=dv=64)
  a  = (qw . kw)/8 - (1-v_mask)*1e10 - tril(ones)*1e10
  o  = softmax(a) @ vw, then o *= q_mask

Sharding: core c in [0,8): batch b = c//4, head-group g = c%4 (4 heads each).
Each core computes o[b, :, 256g:256g+256]; host gathers.

v2 changes vs baseline:
  - fine-grained causal skip: for q-block j and k-chunk c = 4j+dd (dd<4)
    only the first W = 128*(dd+1) q-columns are visible; scores/exp/PV
    all run at width W. PV accumulators pre-zeroed (gpsimd memset) so
    ascending chunks accumulate with start=False.
  - local [128,2,128] strict-lower-triangle mask (one DVE mul per diag
    chunk, both heads at once) instead of 4 wide 512-col masks.
  - prologue: k-proj then q-proj(es0,j0) then attention starts; the
    remaining projections are emitted as fillers interleaved into the
    attention stream (deadline-scheduled).
  - inputs DMA'd in column pieces ordered so compute starts ~4us in.
  - one PSUM ring: tag "st" [128,2,512]f32 x3 (scores, proj psums,
    transpose batches) + tag "ot" [66,512]f32 x2 (PV accumulators).
  - output drained per (es, qb) column-half as soon as each half is done.
"""

import numpy as np

B, L, D = 2, 2048, 1024
H, DK = 16, 64
HG = 4            # heads per core
E = HG * DK       # 256 per-core output features
NCORES = 8
J, QB = 4, 512    # q blocks
C, KB = 16, 128   # k chunks
BIG = 1e10

_CACHE = {}
PROFILE = False
LAST_EXEC_NS = None
LAST_TRACE = None


def _build_program(nd, degen_qbs):
    import concourse.bass as bass
    import concourse.mybir as mybir
    from concourse import bacc
    from concourse.tile import TileContext

    F32 = mybir.dt.float32
    BF16 = mybir.dt.bfloat16
    AF = mybir.ActivationFunctionType
    ALU = mybir.AluOpType
    ts = bass.ts

    nc = bacc.Bacc(None)
    xq = nc.dram_tensor("xq", [D, L], BF16, kind="ExternalInput")
    xk = nc.dram_tensor("xk", [D, L], BF16, kind="ExternalInput")
    xv = nc.dram_tensor("xv", [D, L], BF16, kind="ExternalInput")
    wq = nc.dram_tensor("wq", [128, 8 * E], BF16, kind="ExternalInput")
    wk = nc.dram_tensor("wk", [128, 8 * E], BF16, kind="ExternalInput")
    wv = nc.dram_tensor("wv", [128, 8 * E], BF16, kind="ExternalInput")
    idn = nc.dram_tensor("idn", [128, 128], BF16, kind="ExternalInput")
    oz4 = nc.dram_tensor("oz4", [128, HG, 2], BF16, kind="ExternalInput")
    t01 = nc.dram_tensor("t01", [128, 2, 128], BF16, kind="ExternalInput")
    vmb = nc.dram_tensor("vmb", [128, C], F32, kind="ExternalInput")
    qmt = nc.dram_tensor("qmt", [128, C], F32, kind="ExternalInput")
    isdt = nc.dram_tensor("isdt", [128, C], F32, kind="ExternalInput")
    isdq = nc.dram_tensor("isdq", [128, C, nd], F32, kind="ExternalInput")
    wfx = nc.dram_tensor("wfx", [128, C, 2 * nd], BF16, kind="ExternalInput")
    o_d = nc.dram_tensor("o", [128, C, E], F32, kind="ExternalOutput")

    with TileContext(nc) as tc:
        with tc.tile_pool(name="consts", bufs=1) as consts, \
             tc.tile_pool(name="xk_p", bufs=8) as xkp, \
             tc.tile_pool(name="xq_p", bufs=8) as xqp, \
             tc.tile_pool(name="xv_p", bufs=8) as xvp, \
             tc.tile_pool(name="qk2", bufs=1) as qk2p, \
             tc.tile_pool(name="pp", bufs=8) as ppool, \
             tc.tile_pool(name="osb", bufs=2) as osbp, \
             tc.tile_pool(name="oall", bufs=1) as oallp, \
             tc.tile_pool(name="small", bufs=4) as small, \
             tc.tile_pool(name="fbp", bufs=1) as fbp, \
             tc.tile_pool(name="ps", bufs=1, space="PSUM") as psp:

            # x tiles + weights, DMA'd in compute-feed order
            xt_k = [xkp.tile([128, L], BF16, tag="xk", name=f"xtk_{d}")
                    for d in range(8)]
            xt_q = [xqp.tile([128, L], BF16, tag="xq", name=f"xtq_{d}")
                    for d in range(8)]
            xt_v = [xvp.tile([128, L], BF16, tag="xv", name=f"xtv_{d}")
                    for d in range(8)]
            wsb = {}

            def dma_w(nm):
                wdram = {"k": wk, "q": wq, "v": wv}[nm]
                wsb[nm] = consts.tile([128, 8, E], BF16, tag=f"w_{nm}",
                                      name=f"wsb_{nm}")
                nc.sync.dma_start(
                    out=wsb[nm][:, :, :],
                    in_=wdram.rearrange("p (t e) -> p t e", t=8))

            def dma_x(xin, tiles, half):
                for d in range(8):
                    nc.sync.dma_start(out=tiles[d][:, ts(half, 1024)],
                                      in_=xin[ts(d, 128), ts(half, 1024)])

            dma_w("k")
            dma_x(xk, xt_k, 0)                # xk cols 0:1024 (k-proj lc 0,1)
            dma_w("q")
            dma_x(xq, xt_q, 0)                # xq first half (q-proj j0, j1)

            idt = consts.tile([128, 128], BF16, tag="idt")
            nc.sync.dma_start(out=idt[:, :], in_=idn[:, :])
            t01t = consts.tile([128, 2, 128], BF16, tag="t01t")
            nc.sync.dma_start(out=t01t[:, :, :], in_=t01[:, :, :])
            vmbt = consts.tile([128, C], F32, tag="vmbt")
            nc.sync.dma_start(out=vmbt[:, :], in_=vmb[:, :])
            qmtt = consts.tile([128, C], F32, tag="qmtt")
            nc.sync.dma_start(out=qmtt[:, :], in_=qmt[:, :])
            oz4t = consts.tile([128, HG, 2], BF16, tag="oz4t")
            nc.sync.dma_start(out=oz4t[:, :, :], in_=oz4[:, :, :])

            dma_x(xk, xt_k, 1)                # xk cols 1024: (k02/k03 filler)
            dma_w("v")
            dma_x(xv, xt_v, 0)                # xv first half (vproj lt 0-7)
            dma_x(xq, xt_q, 1)
            dma_x(xv, xt_v, 1)

            # late consts (first needed at yield >= 15)
            isdtt = consts.tile([128, C], F32, tag="isdtt")
            nc.sync.dma_start(out=isdtt[:, :], in_=isdt[:, :])
            isdqt = consts.tile([128, C, nd], F32, tag="isdqt")
            nc.sync.dma_start(out=isdqt[:, :, :], in_=isdq[:, :, :])
            wfxt = consts.tile([128, C, 2 * nd], BF16, tag="wfxt")
            nc.sync.dma_start(out=wfxt[:, :, :], in_=wfx[:, :, :])

            # ---------------- persistent activation tiles ---------------------
            qw2 = [[qk2p.tile([128, QB], BF16, tag=f"qw2_{es}_{lc}",
                              name=f"qw2_{es}_{lc}") for lc in range(4)]
                   for es in range(2)]
            kw2 = [[qk2p.tile([128, QB], BF16, tag=f"kw2_{es}_{lc}",
                              name=f"kw2_{es}_{lc}") for lc in range(4)]
                   for es in range(2)]
            vw_c = [qk2p.tile([128, HG, 66], BF16, tag=f"vw_{c}",
                              name=f"vw_{c}") for c in range(C)]
            oallB = oallp.tile([128, C, E], F32, tag="oall", name="oallB")
            oall = [oallB[:, qb, :] for qb in range(C)]

            # ---------------- proj emitters (psum from shared "st" ring) ------
            def emit_qkproj(nm, es, lc, on_scalar):
                """one (es, lc) projection group: 8 matmuls + copy out."""
                ps = psp.tile([128, QB], F32, tag="pr", bufs=2,
                              name=f"pr_{nm}_{es}_{lc}")
                xt = xt_k if nm == "k" else xt_q
                for d in range(8):
                    nc.tensor.matmul(
                        ps, wsb[nm][:, d, ts(es, 128)],
                        xt[d][:, ts(lc, QB)],
                        start=(d == 0), stop=(d == 7))
                dst = (kw2 if nm == "k" else qw2)[es][lc]
                eng = nc.scalar if on_scalar else nc.vector
                if on_scalar:
                    eng.copy(out=dst[:, :], in_=ps)
                else:
                    eng.tensor_copy(out=dst[:, :], in_=ps)

            def emit_vproj(lt):
                slot = psp.tile([128, QB], F32, tag="pr", bufs=2,
                                name=f"vp_{lt}")
                ps = slot[:, 0:E]
                for d in range(8):
                    nc.tensor.matmul(
                        ps, xt_v[d][:, ts(lt, 128)], wsb["v"][:, d, :],
                        start=(d == 0), stop=(d == 7))
                nc.vector.tensor_copy(
                    out=vw_c[lt][:, :, 0:64],
                    in_=ps.rearrange("p (h e) -> p h e", h=HG))
                nc.sync.dma_start(out=vw_c[lt][:, :, 64:66], in_=oz4t[:, :, :])

            def emit_fix():
                fb = [[None] * nd for _ in range(HG)]
                for i in range(nd):
                    for h in range(HG):
                        pf = psp.tile([2, 64], F32, tag="pr", bufs=2,
                                      name=f"pf_{i}_{h}")
                        for c in range(C):
                            nc.tensor.matmul(
                                pf[:, :],
                                wfxt[:, c, 2 * i:2 * i + 2],
                                vw_c[c][:, h, 0:64],
                                start=(c == 0), stop=(c == C - 1))
                        fr = small.tile([1, 64], F32, tag="fixrow")
                        nc.vector.tensor_copy(out=fr[:, :], in_=pf[0:1, :])
                        t = fbp.tile([128, 64], F32, tag=f"fb_{h}_{i}")
                        nc.gpsimd.partition_broadcast(t[:, :], fr[0:1, :])
                        fb[h][i] = t
                return fb

            # two persistent p0 tiles for each block's first chunk: tails
            # [128:QB) are zeroed once and never rewritten (per-block exp
            # only writes cols [0:128)), so the first PV matmul can run
            # full-width with start=True and zero the whole accumulator
            # bank. Persistent tiles keep all ordering on one logical tile.
            p0_tiles = [qk2p.tile([128, 2, QB], BF16, tag=f"p0_{i}",
                                  name=f"p0_{i}") for i in range(2)]
            for i in range(2):
                nc.vector.memset(p0_tiles[i][:, :, :], 0.0)
            blk_counter = [0]

            # ---------------- attention stream -------------------------------
            LAG = 5
            fb_holder = {}

            def attention_stream():
                for es in range(2):
                    for j in range(J):
                        chunks = list(range(4 * j, C))
                        m = len(chunks)
                        wid = [min(128 * (c - 4 * j + 1), QB) for c in chunks]
                        ot2 = [psp.tile([66, QB], F32, tag="ot", bufs=2,
                                        name=f"ot_{es}_{j}_{s2}")
                               for s2 in range(2)]
                        pbuf = [None] * m

                        def emit_ot(idx, ot2=ot2, pbuf=pbuf, m=m, j=j, es=es,
                                    chunks=chunks, wid=wid):
                            c, w = chunks[idx], wid[idx]
                            last = idx == m - 1
                            for sub in range(2):
                                vws = vw_c[c][:, 2 * es + sub, :]
                                if idx == 0:
                                    # first chunk: full-width start=True;
                                    # pbuf is the p0 tile whose tail
                                    # [128:QB] is permanently zero, so
                                    # cols >= 128 get zeroed for the
                                    # later accumulating chunks.
                                    nc.tensor.matmul(
                                        ot2[sub][:, :], vws,
                                        pbuf[0][:, sub, :],
                                        start=True, stop=False,
                                        skip_group_check=True)
                                else:
                                    nc.tensor.matmul(
                                        ot2[sub][:, 0:w], vws,
                                        pbuf[idx][:, sub, 0:w],
                                        start=False, stop=last,
                                        skip_group_check=True)

                        for idx, c in enumerate(chunks):
                            w = wid[idx]
                            st2 = psp.tile([128, 2, QB], F32, tag="st", bufs=2,
                                           name=f"st_{es}_{j}_{c}")
                            for sub in range(2):
                                r0 = 64 * sub
                                nc.tensor.matmul(
                                    st2[:, sub, 0:w],
                                    kw2[es][c // 4][r0:r0 + 64, ts(c % 4, 128)],
                                    qw2[es][j][r0:r0 + 64, 0:w],
                                    start=True, stop=True)
                            if idx == 0:
                                p = p0_tiles[blk_counter[0] % 2]
                                blk_counter[0] += 1
                            else:
                                p = ppool.tile([128, 2, QB], BF16, tag="p")
                            nc.scalar.activation(
                                out=p[:, :, 0:w], in_=st2[:, :, 0:w],
                                func=AF.Exp,
                                bias=vmbt[:, c:c + 1], scale=0.125)
                            dd = c - 4 * j
                            if dd < 4:
                                off = w - 128
                                nc.vector.tensor_mul(
                                    p[:, :, off:off + 128],
                                    p[:, :, off:off + 128],
                                    t01t[:, :, :])
                            pbuf[idx] = p
                            if idx >= LAG:
                                emit_ot(idx - LAG)
                            yield
                        for idx in range(max(0, m - LAG), m):
                            emit_ot(idx)

                        for sub in range(2):
                            h = 2 * es + sub
                            osb = osbp.tile([66, QB], BF16, tag="osb")
                            nc.vector.tensor_copy(out=osb[:, :],
                                                  in_=ot2[sub][:, :])
                            for t in range(4):
                                qb = 4 * j + t
                                tr = psp.tile([128, 66], BF16, tag="pr",
                                              bufs=2, name=f"tr_{es}_{j}_{sub}_{t}")
                                nc.tensor.transpose(
                                    tr, osb[:, ts(t, 128)], idt[0:66, 0:66])
                                rc = small.tile([128, 1], F32, tag="rc")
                                if qb in degen_qbs:
                                    dn = small.tile([128, 1], F32, tag="dn")
                                    nc.vector.tensor_add(
                                        dn[:, :], tr[:, 64:65],
                                        isdtt[:, qb:qb + 1])
                                    nc.vector.reciprocal(rc[:, :], dn[:, :])
                                else:
                                    nc.vector.reciprocal(rc[:, :], tr[:, 64:65])
                                if es == 1 and j >= 2:
                                    # tail blocks: scale on the (now idle)
                                    # scalar engine; DVE is the tail chain
                                    rc2 = small.tile([128, 1], F32, tag="rc2")
                                    nc.vector.tensor_mul(
                                        rc2[:, :], rc[:, :],
                                        qmtt[:, qb:qb + 1])
                                    nc.scalar.activation(
                                        out=oall[qb][:, ts(h, 64)],
                                        in_=tr[:, 0:64], func=AF.Copy,
                                        scale=rc2[:, 0:1])
                                else:
                                    nc.vector.tensor_scalar(
                                        out=oall[qb][:, ts(h, 64)],
                                        in0=tr[:, 0:64], scalar1=rc[:, 0:1],
                                        scalar2=qmtt[:, qb:qb + 1],
                                        op0=ALU.mult, op1=ALU.mult)
                                for i in degen_qbs.get(qb, ()):
                                    fb = fb_holder["fb"]
                                    nc.vector.scalar_tensor_tensor(
                                        out=oall[qb][:, ts(h, 64)],
                                        in0=fb[h][i][:, :],
                                        scalar=isdqt[:, qb, i:i + 1],
                                        in1=oall[qb][:, ts(h, 64)],
                                        op0=ALU.mult, op1=ALU.add)
                            if sub == 1 and es == 1:
                                nc.sync.dma_start(
                                    out=o_d[:, 4 * j:4 * j + 4, :],
                                    in_=oallB[:, 4 * j:4 * j + 4, :])
                        yield

            # ---------------- prologue + drive -------------------------------
            # prologue: k-proj lc0/lc1 (xk first half) + q-proj (0,0); the
            # rest of the projections are fillers inside the attention stream.
            emit_qkproj("k", 0, 0, on_scalar=True)
            emit_qkproj("k", 0, 1, on_scalar=True)
            emit_qkproj("q", 0, 0, on_scalar=True)

            F = lambda nm, es, lc, sc=False: (
                lambda: emit_qkproj(nm, es, lc, on_scalar=sc))
            # yield-indexed schedule. yields: chunks + 1 epilogue per block;
            # block starts: es0 j0@0 j1@17 j2@30 j3@39; es1 j0@44 j1@61
            # j2@74 j3@83. q-es1 fillers deferred into exp-bound es1 blocks.
            sched = {
                0: [F("q", 0, 1, True)],
                2: [F("k", 0, 2, True)], 4: [F("k", 0, 3, True)],
                6: [F("k", 1, 0, True)], 8: [F("k", 1, 1, True)],
                10: [F("k", 1, 2)], 12: [F("k", 1, 3)],
                14: [F("q", 0, 2)], 17: [F("q", 0, 3)],
                20: [F("q", 1, 0)],
                46: [F("q", 1, 1)], 63: [F("q", 1, 2)], 76: [F("q", 1, 3)],
            }
            # the fix tables (fb) are read at the es0 epilogue of any block
            # holding a degenerate row; emit_fix needs all 16 vw_c tiles.
            vdelay = 0
            # vproj(lt) at yield lt+vdelay (matches xv DMA arrival); PV(c)
            # needs vw_c at yield c+LAG
            for lt in range(C):
                sched.setdefault(lt + vdelay, []).append(
                    (lambda l: lambda: emit_vproj(l))(lt))
            sched.setdefault(C - 1 + vdelay + 1, []).append(
                lambda: fb_holder.update(fb=emit_fix()))

            stream = attention_stream()
            y = 0
            while True:
                for fn in sched.pop(y, ()):
                    fn()
                if next(stream, StopIteration) is StopIteration:
                    break
                y += 1
            for yy in sorted(sched):
                for fn in sched[yy]:
                    fn()
    nc.finalize()
    return nc


def _host_prep(q, k, v, v_mask, q_mask, Wq, Wk, Wv):
    """Per-core input maps + degenerate-row bookkeeping."""
    import ml_dtypes
    bf16 = ml_dtypes.bfloat16
    f32 = np.float32
    q, k, v = (np.asarray(x, f32) for x in (q, k, v))
    v_mask, q_mask = np.asarray(v_mask, f32), np.asarray(q_mask, f32)
    Wq, Wk, Wv = (np.asarray(x, f32) for x in (Wq, Wk, Wv))

    idn = np.eye(128, dtype=f32)
    oz4 = np.zeros((128, HG, 2), f32)
    oz4[:, :, 0] = 1.0
    # local strict-lower-triangle keep mask: keep x_local < p, dup'd on dim1
    p_i = np.arange(128)[:, None]
    x_i = np.arange(128)[None, :]
    t01 = np.where(x_i < p_i, f32(1.0), f32(0.0))
    t01 = np.broadcast_to(t01[:, None, :], (128, 2, 128)).copy()

    # degenerate rows per batch + fix weight vectors
    deg, wfix_cols = [], []
    for b in range(B):
        vm = v_mask[b]
        rows = [qq for qq in range(L)
                if qq == L - 1 or not vm[qq + 1:].any()]
        deg.append(rows)
        cols = []
        for qq in rows:
            single = np.zeros(L, f32)
            kk = np.arange(L)
            causal = kk <= qq
            pen = causal.astype(np.int64) + (vm == 0).astype(np.int64)
            m = pen == pen.min()   # max-attaining set under -BIG penalties
            single[m] = 1.0 / m.sum()
            cols.append(single)
        wfix_cols.append(cols)

    nd = max(len(r) for r in deg)
    degen_qbs = {}
    for b in range(B):
        for i, qq in enumerate(deg[b]):
            degen_qbs.setdefault(qq // 128, set()).add(i)
    degen_qbs = {qb: sorted(s) for qb, s in degen_qbs.items()}

    WqT, WkT, WvT = Wq.T.copy(), Wk.T.copy(), Wv.T.copy()
    in_maps = []
    for core in range(NCORES):
        b, g = divmod(core, HG)
        sl = slice(E * g, E * g + E)
        vm, qm = v_mask[b], q_mask[b]
        vmb = (-BIG * (1.0 - vm)).reshape(C, 128).T.astype(f32)
        qmt = qm.reshape(C, 128).T.astype(f32)
        isd_v = np.zeros(L, f32)
        isdq_v = np.zeros((L, nd), f32)
        wfx_v = np.zeros((L, 2 * nd), f32)
        for i, qq in enumerate(deg[b]):
            isd_v[qq] = 1.0
            isdq_v[qq, i] = qm[qq]
            wfx_v[:, 2 * i] = wfix_cols[b][i]
        in_maps.append({
            "xq": np.ascontiguousarray(q[b].T.astype(bf16)),
            "xk": np.ascontiguousarray(k[b].T.astype(bf16)),
            "xv": np.ascontiguousarray(v[b].T.astype(bf16)),
            # dense pack [128, 8*E]: row p holds [d-chunk, out-feature]
            "wq": np.ascontiguousarray(
                WqT[:, sl].reshape(8, 128, E).transpose(1, 0, 2)
                .reshape(128, 8 * E).astype(bf16)),
            "wk": np.ascontiguousarray(
                WkT[:, sl].reshape(8, 128, E).transpose(1, 0, 2)
                .reshape(128, 8 * E).astype(bf16)),
            "wv": np.ascontiguousarray(
                WvT[:, sl].reshape(8, 128, E).transpose(1, 0, 2)
                .reshape(128, 8 * E).astype(bf16)),
            "idn": idn.astype(bf16),
            "oz4": oz4.astype(bf16),
            "t01": t01.astype(bf16),
            "vmb": vmb, "qmt": qmt,
            "isdt": isd_v.reshape(C, 128).T.copy(),
            "isdq": np.ascontiguousarray(
                isdq_v.reshape(C, 128, nd).transpose(1, 0, 2)),
            "wfx": np.ascontiguousarray(
                wfx_v.reshape(C, 128, 2 * nd).transpose(1, 0, 2).astype(bf16)),
        })
    return in_maps, nd, degen_qbs


def kernel(q, k, v, v_mask, q_mask, Wq, Wk, Wv):
    global LAST_EXEC_NS, LAST_TRACE
    from concourse.bass_utils import run_bass_kernel_spmd

    in_maps, nd, degen_qbs = _host_prep(q, k, v, v_mask, q_mask, Wq, Wk, Wv)
    key = (nd,
           tuple(sorted((qb, tuple(i)) for qb, i in degen_qbs.items())))
    if key not in _CACHE:
        _CACHE[key] = _build_program(nd, degen_qbs)
    nc = _CACHE[key]

    kwargs = {}
    if PROFILE:
        import sys, types
        sys.path.insert(0, "/root/.axon_site/trn_agent_boot")
        import trn_boot
        raw = trn_boot._ntff_profile_via_ctypes("/opt/axon/libaxon_pjrt.so")
        mod = types.ModuleType("antenv.axon_hooks")
        mod.get_axon_ntff_profile_hook = (
            lambda: (lambda out_dir, ids: raw(out_dir, None)))
        sys.modules["antenv.axon_hooks"] = mod
        kwargs = dict(trace=True)

    res = run_bass_kernel_spmd(nc, in_maps, core_ids=list(range(NCORES)), **kwargs)
    if PROFILE:
        LAST_EXEC_NS = res.exec_time_ns
        LAST_TRACE = (res.instructions_and_trace[1]
                      if res.instructions_and_trace else None)

    out = np.empty((B, L, H * DK), np.float32)
    for core in range(NCORES):
        b, g = divmod(core, HG)
        o3 = res.results[core]["o"]
        out[b, :, E * g:E * g + E] = o3.transpose(1, 0, 2).reshape(L, E)
    return out


# revision 10
# speedup vs baseline: 1.2242x; 1.0666x over previous
"""Trainium2 Bass kernel for nn_Attention_68676527063657 (v2).

Full multi-head attention with anti-causal mask (visible iff k > q):
  qw = q @ Wq.T; kw = k @ Wk.T; vw = v @ Wv.T   (per-head, dk=dv=64)
  a  = (qw . kw)/8 - (1-v_mask)*1e10 - tril(ones)*1e10
  o  = softmax(a) @ vw, then o *= q_mask

Sharding: core c in [0,8): batch b = c//4, head-group g = c%4 (4 heads each).
Each core computes o[b, :, 256g:256g+256]; host gathers.

v2 changes vs baseline:
  - fine-grained causal skip: for q-block j and k-chunk c = 4j+dd (dd<4)
    only the first W = 128*(dd+1) q-columns are visible; scores/exp/PV
    all run at width W. PV accumulators pre-zeroed (gpsimd memset) so
    ascending chunks accumulate with start=False.
  - local [128,2,128] strict-lower-triangle mask (one DVE mul per diag
    chunk, both heads at once) instead of 4 wide 512-col masks.
  - prologue: k-proj then q-proj(es0,j0) then attention starts; the
    remaining projections are emitted as fillers interleaved into the
    attention stream (deadline-scheduled).
  - inputs DMA'd in column pieces ordered so compute starts ~4us in.
  - one PSUM ring: tag "st" [128,2,512]f32 x3 (scores, proj psums,
    transpose batches) + tag "ot" [66,512]f32 x2 (PV accumulators).
  - output drained per (es, qb) column-half as soon as each half is done.
"""

import numpy as np

B, L, D = 2, 2048, 1024
H, DK = 16, 64
HG = 4            # heads per core
E = HG * DK       # 256 per-core output features
NCORES = 8
J, QB = 4, 512    # q blocks
C, KB = 16, 128   # k chunks
BIG = 1e10

_CACHE = {}
PROFILE = False
LAST_EXEC_NS = None
LAST_TRACE = None


def _build_program(nd, degen_qbs):
    import concourse.bass as bass
    import concourse.mybir as mybir
    from concourse import bacc
    from concourse.tile import TileContext

    F32 = mybir.dt.float32
    BF16 = mybir.dt.bfloat16
    AF = mybir.ActivationFunctionType
    ALU = mybir.AluOpType
    ts = bass.ts

    nc = bacc.Bacc(None)
    xq = nc.dram_tensor("xq", [D, L], BF16, kind="ExternalInput")
    xk = nc.dram_tensor("xk", [D, L], BF16, kind="ExternalInput")
    xv = nc.dram_tensor("xv", [D, L], BF16, kind="ExternalInput")
    wq = nc.dram_tensor("wq", [128, 8 * E], BF16, kind="ExternalInput")
    wk = nc.dram_tensor("wk", [128, 8 * E], BF16, kind="ExternalInput")
    wv = nc.dram_tensor("wv", [128, 8 * E], BF16, kind="ExternalInput")
    idn = nc.dram_tensor("idn", [128, 128], BF16, kind="ExternalInput")
    oz4 = nc.dram_tensor("oz4", [128, HG, 2], BF16, kind="ExternalInput")
    t01 = nc.dram_tensor("t01", [128, 2, 128], BF16, kind="ExternalInput")
    vmb = nc.dram_tensor("vmb", [128, C], F32, kind="ExternalInput")
    qmt = nc.dram_tensor("qmt", [128, C], F32, kind="ExternalInput")
    isdt = nc.dram_tensor("isdt", [128, C], F32, kind="ExternalInput")
    isdq = nc.dram_tensor("isdq", [128, C, nd], F32, kind="ExternalInput")
    wfx = nc.dram_tensor("wfx", [128, C, 2 * nd], BF16, kind="ExternalInput")
    o_d = nc.dram_tensor("o", [L, E], F32, kind="ExternalOutput")

    with TileContext(nc) as tc:
        with tc.tile_pool(name="consts", bufs=1) as consts, \
             tc.tile_pool(name="xk_p", bufs=8) as xkp, \
             tc.tile_pool(name="xq_p", bufs=8) as xqp, \
             tc.tile_pool(name="xv_p", bufs=8) as xvp, \
             tc.tile_pool(name="qk2", bufs=1) as qk2p, \
             tc.tile_pool(name="pp", bufs=8) as ppool, \
             tc.tile_pool(name="osb", bufs=2) as osbp, \
             tc.tile_pool(name="oall", bufs=1) as oallp, \
             tc.tile_pool(name="small", bufs=4) as small, \
             tc.tile_pool(name="fbp", bufs=1) as fbp, \
             tc.tile_pool(name="ps", bufs=1, space="PSUM") as psp:

            # x tiles + weights, DMA'd in compute-feed order
            xt_k = [xkp.tile([128, L], BF16, tag="xk", name=f"xtk_{d}")
                    for d in range(8)]
            xt_q = [xqp.tile([128, L], BF16, tag="xq", name=f"xtq_{d}")
                    for d in range(8)]
            xt_v = [xvp.tile([128, L], BF16, tag="xv", name=f"xtv_{d}")
                    for d in range(8)]
            wsb = {}

            def dma_w(nm):
                wdram = {"k": wk, "q": wq, "v": wv}[nm]
                wsb[nm] = consts.tile([128, 8, E], BF16, tag=f"w_{nm}",
                                      name=f"wsb_{nm}")
                nc.sync.dma_start(
                    out=wsb[nm][:, :, :],
                    in_=wdram.rearrange("p (t e) -> p t e", t=8))

            def dma_x(xin, tiles, half):
                for d in range(8):
                    nc.sync.dma_start(out=tiles[d][:, ts(half, 1024)],
                                      in_=xin[ts(d, 128), ts(half, 1024)])

            dma_w("k")
            dma_w("v")
            dma_x(xk, xt_k, 0)                # xk cols 0:1024 (k-proj lc 0,1)
            dma_x(xv, xt_v, 0)                # xv first half: vproj lt 0-7
                                              # must not stall the PE queue
            dma_w("q")
            dma_x(xq, xt_q, 0)                # xq first half (q-proj j0, j1)

            idt = consts.tile([128, 128], BF16, tag="idt")
            nc.sync.dma_start(out=idt[:, :], in_=idn[:, :])
            t01t = consts.tile([128, 2, 128], BF16, tag="t01t")
            nc.sync.dma_start(out=t01t[:, :, :], in_=t01[:, :, :])
            vmbt = consts.tile([128, C], F32, tag="vmbt")
            nc.sync.dma_start(out=vmbt[:, :], in_=vmb[:, :])
            qmtt = consts.tile([128, C], F32, tag="qmtt")
            nc.sync.dma_start(out=qmtt[:, :], in_=qmt[:, :])
            oz4t = consts.tile([128, HG, 2], BF16, tag="oz4t")
            nc.sync.dma_start(out=oz4t[:, :, :], in_=oz4[:, :, :])

            dma_x(xk, xt_k, 1)                # xk cols 1024: (k02/k03 filler)
            dma_x(xv, xt_v, 1)
            dma_x(xq, xt_q, 1)

            # late consts (first needed at yield >= 15)
            isdtt = consts.tile([128, C], F32, tag="isdtt")
            nc.sync.dma_start(out=isdtt[:, :], in_=isdt[:, :])
            isdqt = consts.tile([128, C, nd], F32, tag="isdqt")
            nc.sync.dma_start(out=isdqt[:, :, :], in_=isdq[:, :, :])
            wfxt = consts.tile([128, C, 2 * nd], BF16, tag="wfxt")
            nc.sync.dma_start(out=wfxt[:, :, :], in_=wfx[:, :, :])

            # ---------------- persistent activation tiles ---------------------
            qw2 = [[qk2p.tile([128, QB], BF16, tag=f"qw2_{es}_{lc}",
                              name=f"qw2_{es}_{lc}") for lc in range(4)]
                   for es in range(2)]
            kw2 = [[qk2p.tile([128, QB], BF16, tag=f"kw2_{es}_{lc}",
                              name=f"kw2_{es}_{lc}") for lc in range(4)]
                   for es in range(2)]
            vw_c = [qk2p.tile([128, HG, 66], BF16, tag=f"vw_{c}",
                              name=f"vw_{c}") for c in range(C)]
            oallB = oallp.tile([128, C, E], F32, tag="oall", name="oallB")
            oall = [oallB[:, qb, :] for qb in range(C)]

            # ---------------- proj emitters (psum from shared "st" ring) ------
            def emit_qkproj(nm, es, lc, on_scalar):
                """one (es, lc) projection group: 8 matmuls + copy out."""
                ps = psp.tile([128, QB], F32, tag="pr", bufs=2,
                              name=f"pr_{nm}_{es}_{lc}")
                xt = xt_k if nm == "k" else xt_q
                for d in range(8):
                    nc.tensor.matmul(
                        ps, wsb[nm][:, d, ts(es, 128)],
                        xt[d][:, ts(lc, QB)],
                        start=(d == 0), stop=(d == 7))
                dst = (kw2 if nm == "k" else qw2)[es][lc]
                eng = nc.scalar if on_scalar else nc.vector
                if on_scalar:
                    eng.copy(out=dst[:, :], in_=ps)
                else:
                    eng.tensor_copy(out=dst[:, :], in_=ps)

            def emit_vproj(lt):
                slot = psp.tile([128, QB], F32, tag="pr", bufs=2,
                                name=f"vp_{lt}")
                ps = slot[:, 0:E]
                for d in range(8):
                    nc.tensor.matmul(
                        ps, xt_v[d][:, ts(lt, 128)], wsb["v"][:, d, :],
                        start=(d == 0), stop=(d == 7))
                nc.vector.tensor_copy(
                    out=vw_c[lt][:, :, 0:64],
                    in_=ps.rearrange("p (h e) -> p h e", h=HG))
                nc.sync.dma_start(out=vw_c[lt][:, :, 64:66], in_=oz4t[:, :, :])

            def emit_fix():
                fb = [[None] * nd for _ in range(HG)]
                for i in range(nd):
                    for h in range(HG):
                        pf = psp.tile([2, 64], F32, tag="pr", bufs=2,
                                      name=f"pf_{i}_{h}")
                        for c in range(C):
                            nc.tensor.matmul(
                                pf[:, :],
                                wfxt[:, c, 2 * i:2 * i + 2],
                                vw_c[c][:, h, 0:64],
                                start=(c == 0), stop=(c == C - 1))
                        fr = small.tile([1, 64], F32, tag="fixrow")
                        nc.vector.tensor_copy(out=fr[:, :], in_=pf[0:1, :])
                        t = fbp.tile([128, 64], F32, tag=f"fb_{h}_{i}")
                        nc.gpsimd.partition_broadcast(t[:, :], fr[0:1, :])
                        fb[h][i] = t
                return fb

            # two persistent p0 tiles for each block's first chunk: tails
            # [128:QB) are zeroed once and never rewritten (per-block exp
            # only writes cols [0:128)), so the first PV matmul can run
            # full-width with start=True and zero the whole accumulator
            # bank. Persistent tiles keep all ordering on one logical tile.
            p0_tiles = [qk2p.tile([128, 2, QB], BF16, tag=f"p0_{i}",
                                  name=f"p0_{i}") for i in range(2)]
            for i in range(2):
                nc.vector.memset(p0_tiles[i][:, :, :], 0.0)
            blk_counter = [0]

            # ---------------- attention stream -------------------------------
            LAG = 5
            fb_holder = {}

            def attention_stream():
                for es in range(2):
                    for j in range(J):
                        chunks = list(range(4 * j, C))
                        m = len(chunks)
                        wid = [min(128 * (c - 4 * j + 1), QB) for c in chunks]
                        ot2 = [psp.tile([66, QB], F32, tag="ot", bufs=2,
                                        name=f"ot_{es}_{j}_{s2}")
                               for s2 in range(2)]
                        pbuf = [None] * m

                        def emit_ot(idx, ot2=ot2, pbuf=pbuf, m=m, j=j, es=es,
                                    chunks=chunks, wid=wid):
                            c, w = chunks[idx], wid[idx]
                            last = idx == m - 1
                            for sub in range(2):
                                vws = vw_c[c][:, 2 * es + sub, :]
                                if idx == 0:
                                    # first chunk: full-width start=True;
                                    # pbuf is the p0 tile whose tail
                                    # [128:QB] is permanently zero, so
                                    # cols >= 128 get zeroed for the
                                    # later accumulating chunks.
                                    nc.tensor.matmul(
                                        ot2[sub][:, :], vws,
                                        pbuf[0][:, sub, :],
                                        start=True, stop=False,
                                        skip_group_check=True)
                                else:
                                    nc.tensor.matmul(
                                        ot2[sub][:, 0:w], vws,
                                        pbuf[idx][:, sub, 0:w],
                                        start=False, stop=last,
                                        skip_group_check=True)

                        for idx, c in enumerate(chunks):
                            w = wid[idx]
                            st2 = psp.tile([128, 2, QB], F32, tag="st", bufs=2,
                                           name=f"st_{es}_{j}_{c}")
                            for sub in range(2):
                                r0 = 64 * sub
                                nc.tensor.matmul(
                                    st2[:, sub, 0:w],
                                    kw2[es][c // 4][r0:r0 + 64, ts(c % 4, 128)],
                                    qw2[es][j][r0:r0 + 64, 0:w],
                                    start=True, stop=True)
                            if idx == 0:
                                p = p0_tiles[blk_counter[0] % 2]
                                blk_counter[0] += 1
                            else:
                                p = ppool.tile([128, 2, QB], BF16, tag="p")
                            nc.scalar.activation(
                                out=p[:, :, 0:w], in_=st2[:, :, 0:w],
                                func=AF.Exp,
                                bias=vmbt[:, c:c + 1], scale=0.125)
                            dd = c - 4 * j
                            if dd < 4:
                                off = w - 128
                                nc.vector.tensor_mul(
                                    p[:, :, off:off + 128],
                                    p[:, :, off:off + 128],
                                    t01t[:, :, :])
                            pbuf[idx] = p
                            if idx >= LAG:
                                emit_ot(idx - LAG)
                            yield
                        for idx in range(max(0, m - LAG), m):
                            emit_ot(idx)

                        for sub in range(2):
                            h = 2 * es + sub
                            osb = osbp.tile([66, QB], BF16, tag="osb")
                            nc.vector.tensor_copy(out=osb[:, :],
                                                  in_=ot2[sub][:, :])
                            for t in range(4):
                                qb = 4 * j + t
                                tr = psp.tile([128, 66], BF16, tag="pr",
                                              bufs=2, name=f"tr_{es}_{j}_{sub}_{t}")
                                nc.tensor.transpose(
                                    tr, osb[:, ts(t, 128)], idt[0:66, 0:66])
                                rc = small.tile([128, 1], F32, tag="rc")
                                if qb in degen_qbs:
                                    dn = small.tile([128, 1], F32, tag="dn")
                                    nc.vector.tensor_add(
                                        dn[:, :], tr[:, 64:65],
                                        isdtt[:, qb:qb + 1])
                                    nc.vector.reciprocal(rc[:, :], dn[:, :])
                                else:
                                    nc.vector.reciprocal(rc[:, :], tr[:, 64:65])
                                nc.vector.tensor_scalar(
                                    out=oall[qb][:, ts(h, 64)],
                                    in0=tr[:, 0:64], scalar1=rc[:, 0:1],
                                    scalar2=qmtt[:, qb:qb + 1],
                                    op0=ALU.mult, op1=ALU.mult)
                                for i in degen_qbs.get(qb, ()):
                                    fb = fb_holder["fb"]
                                    nc.vector.scalar_tensor_tensor(
                                        out=oall[qb][:, ts(h, 64)],
                                        in0=fb[h][i][:, :],
                                        scalar=isdqt[:, qb, i:i + 1],
                                        in1=oall[qb][:, ts(h, 64)],
                                        op0=ALU.mult, op1=ALU.add)
                            if sub == 1:
                                nc.sync.dma_start(
                                    out=o_d[ts(j, QB), ts(es, 128)].rearrange(
                                        "(t p) e -> p t e", p=128),
                                    in_=oallB[:, 4 * j:4 * j + 4, ts(es, 128)])
                        yield

            # ---------------- prologue + drive -------------------------------
            # prologue: k-proj lc0/lc1 (xk first half) + q-proj (0,0); the
            # rest of the projections are fillers inside the attention stream.
            emit_qkproj("k", 0, 0, on_scalar=True)
            emit_qkproj("k", 0, 1, on_scalar=True)
            emit_qkproj("q", 0, 0, on_scalar=True)

            F = lambda nm, es, lc, sc=False: (
                lambda: emit_qkproj(nm, es, lc, on_scalar=sc))
            # yield-indexed schedule. yields: chunks + 1 epilogue per block;
            # block starts: es0 j0@0 j1@17 j2@30 j3@39; es1 j0@44 j1@61
            # j2@74 j3@83. q-es1 fillers deferred into exp-bound es1 blocks.
            sched = {
                0: [F("q", 0, 1, True)],
                6: [F("k", 0, 2, True)], 7: [F("k", 0, 3, True)],
                12: [F("k", 1, 0, True)], 14: [F("k", 1, 1, True)],
                16: [F("k", 1, 2)], 18: [F("k", 1, 3)],
                24: [F("q", 0, 2)], 26: [F("q", 0, 3)],
                28: [F("q", 1, 0)],
                46: [F("q", 1, 1)], 63: [F("q", 1, 2)], 76: [F("q", 1, 3)],
            }
            # the fix tables (fb) are read at the es0 epilogue of any block
            # holding a degenerate row; emit_fix needs all 16 vw_c tiles.
            vdelay = 0
            # vproj(lt) at yield lt+vdelay (matches xv DMA arrival); PV(c)
            # needs vw_c at yield c+LAG
            for lt in range(C):
                sched.setdefault(lt + vdelay, []).append(
                    (lambda l: lambda: emit_vproj(l))(lt))
            sched.setdefault(C - 1 + vdelay + 1, []).append(
                lambda: fb_holder.update(fb=emit_fix()))

            stream = attention_stream()
            y = 0
            while True:
                for fn in sched.pop(y, ()):
                    fn()
                if next(stream, StopIteration) is StopIteration:
                    break
                y += 1
            for yy in sorted(sched):
                for fn in sched[yy]:
                    fn()
    nc.finalize()
    return nc


def _host_prep(q, k, v, v_mask, q_mask, Wq, Wk, Wv):
    """Per-core input maps + degenerate-row bookkeeping."""
    import ml_dtypes
    bf16 = ml_dtypes.bfloat16
    f32 = np.float32
    q, k, v = (np.asarray(x, f32) for x in (q, k, v))
    v_mask, q_mask = np.asarray(v_mask, f32), np.asarray(q_mask, f32)
    Wq, Wk, Wv = (np.asarray(x, f32) for x in (Wq, Wk, Wv))

    idn = np.eye(128, dtype=f32)
    oz4 = np.zeros((128, HG, 2), f32)
    oz4[:, :, 0] = 1.0
    # local strict-lower-triangle keep mask: keep x_local < p, dup'd on dim1
    p_i = np.arange(128)[:, None]
    x_i = np.arange(128)[None, :]
    t01 = np.where(x_i < p_i, f32(1.0), f32(0.0))
    t01 = np.broadcast_to(t01[:, None, :], (128, 2, 128)).copy()

    # degenerate rows per batch + fix weight vectors
    deg, wfix_cols = [], []
    for b in range(B):
        vm = v_mask[b]
        rows = [qq for qq in range(L)
                if qq == L - 1 or not vm[qq + 1:].any()]
        deg.append(rows)
        cols = []
        for qq in rows:
            single = np.zeros(L, f32)
            kk = np.arange(L)
            causal = kk <= qq
            pen = causal.astype(np.int64) + (vm == 0).astype(np.int64)
            m = pen == pen.min()   # max-attaining set under -BIG penalties
            single[m] = 1.0 / m.sum()
            cols.append(single)
        wfix_cols.append(cols)

    nd = max(len(r) for r in deg)
    degen_qbs = {}
    for b in range(B):
        for i, qq in enumerate(deg[b]):
            degen_qbs.setdefault(qq // 128, set()).add(i)
    degen_qbs = {qb: sorted(s) for qb, s in degen_qbs.items()}

    WqT, WkT, WvT = Wq.T.copy(), Wk.T.copy(), Wv.T.copy()
    in_maps = []
    for core in range(NCORES):
        b, g = divmod(core, HG)
        sl = slice(E * g, E * g + E)
        vm, qm = v_mask[b], q_mask[b]
        vmb = (-BIG * (1.0 - vm)).reshape(C, 128).T.astype(f32)
        qmt = qm.reshape(C, 128).T.astype(f32)
        isd_v = np.zeros(L, f32)
        isdq_v = np.zeros((L, nd), f32)
        wfx_v = np.zeros((L, 2 * nd), f32)
        for i, qq in enumerate(deg[b]):
            isd_v[qq] = 1.0
            isdq_v[qq, i] = qm[qq]
            wfx_v[:, 2 * i] = wfix_cols[b][i]
        in_maps.append({
            "xq": np.ascontiguousarray(q[b].T.astype(bf16)),
            "xk": np.ascontiguousarray(k[b].T.astype(bf16)),
            "xv": np.ascontiguousarray(v[b].T.astype(bf16)),
            # dense pack [128, 8*E]: row p holds [d-chunk, out-feature]
            "wq": np.ascontiguousarray(
                WqT[:, sl].reshape(8, 128, E).transpose(1, 0, 2)
                .reshape(128, 8 * E).astype(bf16)),
            "wk": np.ascontiguousarray(
                WkT[:, sl].reshape(8, 128, E).transpose(1, 0, 2)
                .reshape(128, 8 * E).astype(bf16)),
            "wv": np.ascontiguousarray(
                WvT[:, sl].reshape(8, 128, E).transpose(1, 0, 2)
                .reshape(128, 8 * E).astype(bf16)),
            "idn": idn.astype(bf16),
            "oz4": oz4.astype(bf16),
            "t01": t01.astype(bf16),
            "vmb": vmb, "qmt": qmt,
            "isdt": isd_v.reshape(C, 128).T.copy(),
            "isdq": np.ascontiguousarray(
                isdq_v.reshape(C, 128, nd).transpose(1, 0, 2)),
            "wfx": np.ascontiguousarray(
                wfx_v.reshape(C, 128, 2 * nd).transpose(1, 0, 2).astype(bf16)),
        })
    return in_maps, nd, degen_qbs


def kernel(q, k, v, v_mask, q_mask, Wq, Wk, Wv):
    global LAST_EXEC_NS, LAST_TRACE
    from concourse.bass_utils import run_bass_kernel_spmd

    in_maps, nd, degen_qbs = _host_prep(q, k, v, v_mask, q_mask, Wq, Wk, Wv)
    key = (nd,
           tuple(sorted((qb, tuple(i)) for qb, i in degen_qbs.items())))
    if key not in _CACHE:
        _CACHE[key] = _build_program(nd, degen_qbs)
    nc = _CACHE[key]

    kwargs = {}
    if PROFILE:
        import sys, types
        sys.path.insert(0, "/root/.axon_site/trn_agent_boot")
        import trn_boot
        raw = trn_boot._ntff_profile_via_ctypes("/opt/axon/libaxon_pjrt.so")
        mod = types.ModuleType("antenv.axon_hooks")
        mod.get_axon_ntff_profile_hook = (
            lambda: (lambda out_dir, ids: raw(out_dir, None)))
        sys.modules["antenv.axon_hooks"] = mod
        kwargs = dict(trace=True)

    res = run_bass_kernel_spmd(nc, in_maps, core_ids=list(range(NCORES)), **kwargs)
    if PROFILE:
        LAST_EXEC_NS = res.exec_time_ns
        LAST_TRACE = (res.instructions_and_trace[1]
                      if res.instructions_and_trace else None)

    out = np.empty((B, L, H * DK), np.float32)
    for core in range(NCORES):
        b, g = divmod(core, HG)
        out[b, :, E * g:E * g + E] = res.results[core]["o"]
    return out
